# revision 17
# baseline (speedup 1.0000x reference)
"""Distributed Trainium2 kernel for the contrastive InfoNCE loss problem.

Strategy: shard the P = SY*SX = 275 position axis across 8 NeuronCores
(36 position slots per core, zero-padded + weight-masked).  Each core
computes, for its positions p (bf16 matmul pipeline, f32 accumulation):
    pos_p^T  = Wl.T @ locp_p^T + bias_prev^T          [D, N]
    pred_p^T = Wl.T @ loct_p^T + bias_t^T             [D, N]
    logits1^T[m,n] = pos_p[m] . f[n]     (lhsT = pos_p^T tile, rhs = f^T)
    logits2^T[m,n] = pos_p[m] . pred_p[n]
    lse sums via shifted-ones-band matmuls over exp tiles -> one PSUM row
    per position; diagonal terms via a PSUM-accumulated G = sum_p rawpos_p
    (loss1) and a gpsimd multiply + DVE reduce (loss2).
Host sums the 8 per-core scalars and divides by P*N.
"""

import numpy as np

# Problem constants (from the nn_ALL_9320079032780 spec).
N = 256
C = 128
SY, SX = 11, 25
P = SY * SX  # 275
D = 128
DM = 64
DC = 64
N_CORES = 8
POS_PER_CORE = 36  # padded; 18 supers of 2 positions
N_SUPERS = POS_PER_CORE // 2
WBW = 35 + 128  # width of the shifted ones-column band matrix

EXP_SHIFT = 20.0

# packed bf16 statics layout: [Wl | Wmc | fT | mcpT | mctT | wband]
_OFF_WL = 0
_OFF_WMC = _OFF_WL + D
_OFF_FT = _OFF_WMC + D
_OFF_MCP = _OFF_FT + N
_OFF_MCT = _OFF_MCP + N
_OFF_WB = _OFF_MCT + N
STB_COLS = _OFF_WB + WBW
# packed f32 statics layout: [bj | ones | wrow | nposb | wposb]
STF_COLS = 4 + POS_PER_CORE

_CACHED_NC = None


def _build_nc():
    import concourse.bass as bass  # noqa: F401
    import concourse.mybir as mybir
    import concourse.tile as tile
    from concourse import bacc

    f32 = mybir.dt.float32
    bf16 = mybir.dt.bfloat16
    Alu = mybir.AluOpType
    Act = mybir.ActivationFunctionType

    nc = bacc.Bacc("TRN2", target_bir_lowering=False, debug=False,
                   num_devices=N_CORES)

    locp_d = nc.declare_dram_parameter("locp", [N_SUPERS, C, 2 * N], bf16, isOutput=False)
    loct_d = nc.declare_dram_parameter("loct", [N_SUPERS, C, 2 * N], bf16, isOutput=False)
    stb_d = nc.declare_dram_parameter("stb", [128, STB_COLS], bf16, isOutput=False)
    stf_d = nc.declare_dram_parameter("stf", [128, STF_COLS], f32, isOutput=False)
    out_d = nc.declare_dram_parameter("out", [1, 1], f32, isOutput=True)

    with tile.TileContext(nc) as tc:
        with (
            tc.tile_pool(name="statics", bufs=1) as st,
            tc.tile_pool(name="loc", bufs=3) as locpool,
            tc.tile_pool(name="work", bufs=3) as work,
            tc.tile_pool(name="exps", bufs=3) as exps,
            tc.tile_pool(name="pp", bufs=1, space="PSUM") as ppp,
            tc.tile_pool(name="lg", bufs=2, space="PSUM") as lgp,
            tc.tile_pool(name="acc", bufs=1, space="PSUM") as accp,
        ):
            # ---- statics: two packed DMAs ----
            stb = st.tile([128, STB_COLS], bf16, tag="stb")
            stf = st.tile([128, STF_COLS], f32, tag="stf")
            nc.sync.dma_start(out=stb[:, :], in_=stb_d[:, :])
            nc.sync.dma_start(out=stf[:, :], in_=stf_d[:, :])
            Wl = stb[:, _OFF_WL:_OFF_WL + D]
            Wmc = stb[:, _OFF_WMC:_OFF_WMC + D]
            fT = stb[:, _OFF_FT:_OFF_FT + N]
            mcpT = stb[:, _OFF_MCP:_OFF_MCP + N]
            mctT = stb[:, _OFF_MCT:_OFF_MCT + N]
            wband = stb[:, _OFF_WB:_OFF_WB + WBW]
            bj = stf[:, 0:1]
            ones = stf[:, 1:2]
            wrow = stf[:, 2:3]
            nposb = stf[:, 3:4]
            wposb = stf[:, 4:4 + POS_PER_CORE]

            shiftc = st.tile([128, 1], f32, tag="shiftc")
            nc.vector.memset(shiftc[:, :], -EXP_SHIFT)

            # ---- bias matrices: bias^T = Wmc.T @ mcT + bj  ([D, N]) ----
            ps_b = ppp.tile([128, 4 * N], f32, tag="pp")
            nc.tensor.matmul(out=ps_b[:, 0:N], lhsT=Wmc,
                             rhs=mcpT, start=True, stop=False)
            nc.tensor.matmul(out=ps_b[:, N:2 * N], lhsT=Wmc,
                             rhs=mctT, start=False, stop=True)
            biasP = st.tile([D, N], f32, tag="biasP")
            biasT = st.tile([D, N], f32, tag="biasT")
            nc.vector.tensor_scalar_add(biasP[:, :], ps_b[:, 0:N], bj)
            nc.vector.tensor_scalar_add(biasT[:, :], ps_b[:, N:2 * N], bj)
            # doubled copies for per-super adds
            biasP2 = st.tile([D, 2 * N], f32, tag="biasP2")
            biasT2 = st.tile([D, 2 * N], f32, tag="biasT2")
            nc.vector.tensor_copy(biasP2[:, 0:N], biasP[:, :])
            nc.vector.tensor_copy(biasP2[:, N:2 * N], biasP[:, :])
            nc.vector.tensor_copy(biasT2[:, 0:N], biasT[:, :])
            nc.vector.tensor_copy(biasT2[:, N:2 * N], biasT[:, :])

            # persistent accumulators
            psum_S = accp.tile([128, 2 * N], f32, tag="S")   # row j = lse sums
            psum_G = accp.tile([128, N], f32, tag="G")       # sum_p rawpos_p^T
            diag2buf = st.tile([128, POS_PER_CORE], f32, tag="d2buf")

            # ---- software-pipelined producer: loads + pos/pred + bias for
            # super s (G matmuls are emitted separately, off the critical
            # path) ----
            def stage_a(s):
                lp = locpool.tile([C, 2 * N], bf16, tag="lp")
                lt = locpool.tile([C, 2 * N], bf16, tag="lt")
                nc.sync.dma_start(out=lp[:, :], in_=locp_d[s, :, :])
                nc.sync.dma_start(out=lt[:, :], in_=loct_d[s, :, :])

                pp = ppp.tile([128, 4 * N], f32, tag="pp")
                nc.tensor.matmul(out=pp[:, 0:2 * N], lhsT=Wl,
                                 rhs=lp[:, :], start=True, stop=True)
                nc.tensor.matmul(out=pp[:, 2 * N:4 * N], lhsT=Wl,
                                 rhs=lt[:, :], start=True, stop=True)

                posT2 = work.tile([128, 2 * N], bf16, tag="posT2")
                predT2 = work.tile([128, 2 * N], bf16, tag="predT2")
                nc.vector.tensor_tensor(out=posT2[:, :], in0=pp[:, 0:2 * N],
                                        in1=biasP2[:, :], op=Alu.add)
                nc.vector.tensor_tensor(out=predT2[:, :], in0=pp[:, 2 * N:4 * N],
                                        in1=biasT2[:, :], op=Alu.add)
                return posT2, predT2, lp

            def emit_G(s, lp):
                # G accumulation (raw positive, before bias)
                nc.tensor.matmul(out=psum_G[:, :], lhsT=Wl,
                                 rhs=lp[:, 0:N],
                                 start=(s == 0), stop=False)
                nc.tensor.matmul(out=psum_G[:, :], lhsT=Wl,
                                 rhs=lp[:, N:2 * N],
                                 start=False, stop=(s == N_SUPERS - 1))

            def emit_logits(posT2, predT2, co):
                lg = lgp.tile([128, 4 * N], f32, tag="lg")
                nc.tensor.matmul(out=lg[:, 0:N],
                                 lhsT=posT2[:, co:co + 128],
                                 rhs=fT, start=True, stop=False)
                nc.tensor.matmul(out=lg[:, 2 * N:3 * N],
                                 lhsT=posT2[:, co:co + 128],
                                 rhs=predT2[:, co:co + N],
                                 start=True, stop=False)
                nc.tensor.matmul(out=lg[:, N:2 * N],
                                 lhsT=posT2[:, co + 128:co + 256],
                                 rhs=fT, start=False, stop=True)
                nc.tensor.matmul(out=lg[:, 3 * N:4 * N],
                                 lhsT=posT2[:, co + 128:co + 256],
                                 rhs=predT2[:, co:co + N],
                                 start=False, stop=True)
                return lg

            def emit_exp(lg):
                et = exps.tile([128, 4 * N], bf16, tag="et")
                # exp(l - EXP_SHIFT): keeps sums within ScalarE Ln range;
                # the shift is added back on the host (+2*EXP_SHIFT).
                nc.scalar.activation(et[:, :], lg[:, :], Act.Exp,
                                     bias=shiftc[:, 0:1])
                return et

            def emit_lse(j, et):
                # shifted ones-column band: lhsT = wband[:, 35-j : 163-j]
                # has its all-ones column at output row j, accumulating
                # sum_m exp into psum_S[j, :].
                first = (j == 0)
                last = (j == POS_PER_CORE - 1)
                wb = wband[:, 35 - j:35 - j + 128]
                nc.tensor.matmul(out=psum_S[:, 0:N],
                                 lhsT=wb, rhs=et[:, 0:N],
                                 start=first, stop=False)
                nc.tensor.matmul(out=psum_S[:, 0:N],
                                 lhsT=wb, rhs=et[:, N:2 * N],
                                 start=False, stop=False)
                nc.tensor.matmul(out=psum_S[:, N:2 * N],
                                 lhsT=wb, rhs=et[:, 2 * N:3 * N],
                                 start=False, stop=False)
                nc.tensor.matmul(out=psum_S[:, N:2 * N],
                                 lhsT=wb, rhs=et[:, 3 * N:4 * N],
                                 start=False, stop=last)

            staged = stage_a(0)
            emit_G(0, staged[2])
            for s in range(N_SUPERS):
                posT2, predT2, _ = staged

                lg0 = emit_logits(posT2, predT2, 0)
                if s + 1 < N_SUPERS:
                    staged = stage_a(s + 1)
                et0 = emit_exp(lg0)
                emit_lse(2 * s, et0)

                # diag2: d2s = posT*predT (gpsimd), segmented-reduce on DVE
                d2s = work.tile([128, 2 * N], f32, tag="d2s")
                nc.gpsimd.tensor_tensor(out=d2s[:, :], in0=posT2[:, :],
                                        in1=predT2[:, :], op=Alu.mult)
                nc.vector.tensor_reduce(
                    out=diag2buf[:, 2 * s:2 * s + 2],
                    in_=d2s[:, :].rearrange("p (k n) -> p k n", k=2),
                    axis=mybir.AxisListType.X, op=Alu.add)

                lg1 = emit_logits(posT2, predT2, N)
                et1 = emit_exp(lg1)
                emit_lse(2 * s + 1, et1)

                if s + 1 < N_SUPERS:
                    emit_G(s + 1, staged[2])

            # ---- epilogue ----
            J = POS_PER_CORE
            logout = st.tile([128, 2 * N], f32, tag="logout")
            logacc = st.tile([128, 1], f32, tag="logacc")
            nc.scalar.activation(logout[0:J, :], psum_S[0:J, :], Act.Ln,
                                 accum_out=logacc[0:J, 0:1])

            scol = st.tile([128, 4], f32, tag="scol")
            scr1 = st.tile([128, N], f32, tag="scr1")
            # diag1 raw part: <fT, G>
            nc.vector.tensor_tensor(out=scr1[:, :], in0=fT,
                                    in1=psum_G[:, :], op=Alu.mult)
            nc.vector.tensor_reduce(out=scol[:, 0:1], in_=scr1[:, :],
                                    axis=mybir.AxisListType.X, op=Alu.add)
            # diag1 bias part: npos * <fT, biasP>
            scr2 = st.tile([128, N], f32, tag="scr2")
            bcol = st.tile([128, 1], f32, tag="bcol")
            nc.vector.tensor_tensor(out=scr2[:, :], in0=fT,
                                    in1=biasP[:, :], op=Alu.mult)
            nc.vector.tensor_reduce(out=bcol[:, 0:1], in_=scr2[:, :],
                                    axis=mybir.AxisListType.X, op=Alu.add)
            nc.vector.tensor_tensor(out=scol[:, 1:2], in0=bcol[:, :],
                                    in1=nposb, op=Alu.mult)
            # diag2 weighted
            wd2 = st.tile([128, POS_PER_CORE], f32, tag="wd2")
            nc.vector.tensor_tensor(out=wd2[:, :], in0=diag2buf[:, :],
                                    in1=wposb, op=Alu.mult)
            nc.vector.tensor_reduce(out=scol[:, 2:3], in_=wd2[:, :],
                                    axis=mybir.AxisListType.X, op=Alu.add)
            nc.vector.memset(scol[:, 3:4], 0.0)

            # weighted lse total -> psF[64,0]; diag total -> psF[64,1:4]
            psF = lgp.tile([128, 4 * N], f32, tag="lg")
            nc.tensor.matmul(out=psF[64:65, 0:1], lhsT=wrow[0:J, :],
                             rhs=logacc[0:J, :], start=True, stop=False)
            nc.tensor.matmul(out=psF[64:65, 1:4], lhsT=ones,
                             rhs=scol[:, 0:3], start=False, stop=True)
            tmp1 = st.tile([1, 1], f32, tag="tmp1")
            out_sb = st.tile([1, 1], f32, tag="out_sb")
            nc.vector.tensor_reduce(out=tmp1[0:1, :], in_=psF[64:65, 1:4],
                                    axis=mybir.AxisListType.X, op=Alu.add)
            nc.vector.tensor_tensor(out=out_sb[0:1, :], in0=psF[64:65, 0:1],
                                    in1=tmp1[0:1, :], op=Alu.subtract)
            nc.sync.dma_start(out=out_d[:, :], in_=out_sb[0:1, :])

    nc.finalize()
    return nc


def _get_nc():
    global _CACHED_NC
    if _CACHED_NC is None:
        _CACHED_NC = _build_nc()
    return _CACHED_NC


def _core_position_lists():
    """275 positions -> 8 cores: 3 cores x 35, 5 cores x 34."""
    lists = []
    start = 0
    for i in range(N_CORES):
        cnt = 35 if i < 3 else 34
        lists.append(list(range(start, start + cnt)))
        start += cnt
    assert start == P
    return lists


def _prep_in_maps(f_t_global, x_t_local, x_t_prev_local, m_t, m_t_prev, c_t,
                  c_t_prev, W_join, b_join):
    import ml_dtypes
    bf16 = ml_dtypes.bfloat16

    # [N, C, SY, SX] -> [P, C, N]
    locp_full = np.ascontiguousarray(
        x_t_prev_local.reshape(N, C, P).transpose(2, 1, 0))
    loct_full = np.ascontiguousarray(
        x_t_local.reshape(N, C, P).transpose(2, 1, 0))

    # packed bf16 statics [128, STB_COLS]
    stb = np.zeros((128, STB_COLS), dtype=np.float32)
    stb[:, _OFF_WL:_OFF_WL + D] = W_join[:C]
    stb[:, _OFF_WMC:_OFF_WMC + D] = W_join[C:]
    stb[:, _OFF_FT:_OFF_FT + N] = f_t_global.T
    stb[:, _OFF_MCP:_OFF_MCP + N] = np.concatenate([m_t_prev, c_t_prev], 1).T
    stb[:, _OFF_MCT:_OFF_MCT + N] = np.concatenate([m_t, c_t], 1).T
    stb[:, _OFF_WB + 35] = 1.0
    stb = stb.astype(bf16)

    in_maps = []
    for ids in _core_position_lists():
        npos = len(ids)
        locp = np.zeros((POS_PER_CORE, C, N), dtype=np.float32)
        loct = np.zeros((POS_PER_CORE, C, N), dtype=np.float32)
        locp[:npos] = locp_full[ids]
        loct[:npos] = loct_full[ids]
        # [36, C, N] -> [18, C, 2N] (two consecutive positions side by side)
        locp = np.ascontiguousarray(
            locp.reshape(N_SUPERS, 2, C, N).transpose(0, 2, 1, 3)
                .reshape(N_SUPERS, C, 2 * N)).astype(bf16)
        loct = np.ascontiguousarray(
            loct.reshape(N_SUPERS, 2, C, N).transpose(0, 2, 1, 3)
                .reshape(N_SUPERS, C, 2 * N)).astype(bf16)
        # packed f32 statics [128, STF_COLS]: [bj | ones | wrow | nposb | wposb]
        stf = np.zeros((128, STF_COLS), dtype=np.float32)
        stf[:, 0] = b_join.reshape(D)
        stf[:, 1] = 1.0
        stf[:npos, 2] = 1.0
        stf[:, 3] = float(npos)
        stf[:, 4:4 + npos] = 1.0
        in_maps.append({"locp": locp, "loct": loct, "stb": stb, "stf": stf})
    return in_maps


def kernel(f_t_global, x_t_local, x_t_prev_local, m_t, m_t_prev, c_t,
           c_t_prev, W_join, b_join):
    from concourse.bass_utils import run_bass_kernel_spmd

    args = [f_t_global, x_t_local, x_t_prev_local, m_t, m_t_prev, c_t,
            c_t_prev, W_join, b_join]
    args = [np.asarray(a, dtype=np.float32) for a in args]
    in_maps = _prep_in_maps(*args)
    nc = _get_nc()
    res = run_bass_kernel_spmd(nc, in_maps, core_ids=list(range(N_CORES)))
    total = 0.0
    for i in range(N_CORES):
        total += float(res.results[i]["out"][0, 0])
    return np.float32(total / (P * N) + 2.0 * EXP_SHIFT)


# revision 19
# speedup vs baseline: 1.1494x; 1.1494x over previous
"""Distributed Trainium2 kernel for the contrastive InfoNCE loss problem.

Strategy: shard the P = SY*SX = 275 position axis across 8 NeuronCores
(36 position slots per core, zero-padded + weight-masked).  Each core
computes, for its positions p (bf16 matmul pipeline, f32 accumulation):
    pos_p^T  = Wl.T @ locp_p^T + bias_prev^T          [D, N]
    pred_p^T = Wl.T @ loct_p^T + bias_t^T             [D, N]
    logits1^T[m,n] = pos_p[m] . f[n]     (lhsT = pos_p^T tile, rhs = f^T)
    logits2^T[m,n] = pos_p[m] . pred_p[n]
    lse sums via shifted-ones-band matmuls over exp tiles -> one PSUM row
    per position; diagonal terms via a PSUM-accumulated G = sum_p rawpos_p
    (loss1) and a gpsimd multiply + DVE reduce (loss2).
Host sums the 8 per-core scalars and divides by P*N.
"""

import numpy as np

# Problem constants (from the nn_ALL_9320079032780 spec).
N = 256
C = 128
SY, SX = 11, 25
P = SY * SX  # 275
D = 128
DM = 64
DC = 64
N_CORES = 8
POS_PER_CORE = 36  # padded; 18 supers of 2 positions
N_SUPERS = POS_PER_CORE // 2
WBW = 35 + 128  # width of the shifted ones-column band matrix

EXP_SHIFT = 20.0

# packed bf16 statics layout: [Wl | Wmc | fT | mcpT | mctT | wband]
_OFF_WL = 0
_OFF_WMC = _OFF_WL + D
_OFF_FT = _OFF_WMC + D
_OFF_MCP = _OFF_FT + N
_OFF_MCT = _OFF_MCP + N
_OFF_WB = _OFF_MCT + N
STB_COLS = _OFF_WB + WBW
# packed f32 statics layout: [bj | ones | wrow | nposb | wposb]
STF_COLS = 4 + POS_PER_CORE

_CACHED_NC = None


def _build_nc():
    import concourse.bass as bass  # noqa: F401
    import concourse.mybir as mybir
    import concourse.tile as tile
    from concourse import bacc

    f32 = mybir.dt.float32
    bf16 = mybir.dt.bfloat16
    Alu = mybir.AluOpType
    Act = mybir.ActivationFunctionType

    nc = bacc.Bacc("TRN2", target_bir_lowering=False, debug=False,
                   num_devices=N_CORES)

    # Make the act-table pass pick the combined exp+ln set so the kernel
    # pays a single ACT_TABLE_LOAD instead of one per function family.
    from concourse.hw_specs import get_activation_tables
    _tabs = get_activation_tables(nc.m.arch)
    _Exp, _Ln = mybir.ActivationFunctionType.Exp, mybir.ActivationFunctionType.Ln
    for _name, _fns in _tabs.items():
        if _name != "natural_log_exp_and_others":
            _fns.discard(_Exp)
            _fns.discard(_Ln)

    locp_d = nc.declare_dram_parameter("locp", [N_SUPERS, C, 2 * N], bf16, isOutput=False)
    loct_d = nc.declare_dram_parameter("loct", [N_SUPERS, C, 2 * N], bf16, isOutput=False)
    stb_d = nc.declare_dram_parameter("stb", [128, STB_COLS], bf16, isOutput=False)
    stf_d = nc.declare_dram_parameter("stf", [128, STF_COLS], f32, isOutput=False)
    out_d = nc.declare_dram_parameter("out", [1, 1], f32, isOutput=True)

    with tile.TileContext(nc) as tc:
        with (
            tc.tile_pool(name="statics", bufs=1) as st,
            tc.tile_pool(name="loc", bufs=3) as locpool,
            tc.tile_pool(name="work", bufs=3) as work,
            tc.tile_pool(name="exps", bufs=3) as exps,
            tc.tile_pool(name="pp", bufs=1, space="PSUM") as ppp,
            tc.tile_pool(name="lg", bufs=2, space="PSUM") as lgp,
            tc.tile_pool(name="acc", bufs=1, space="PSUM") as accp,
        ):
            # ---- statics: two packed DMAs ----
            stb = st.tile([128, STB_COLS], bf16, tag="stb")
            stf = st.tile([128, STF_COLS], f32, tag="stf")
            nc.sync.dma_start(out=stb[:, :], in_=stb_d[:, :])
            nc.sync.dma_start(out=stf[:, :], in_=stf_d[:, :])
            Wl = stb[:, _OFF_WL:_OFF_WL + D]
            Wmc = stb[:, _OFF_WMC:_OFF_WMC + D]
            fT = stb[:, _OFF_FT:_OFF_FT + N]
            mcpT = stb[:, _OFF_MCP:_OFF_MCP + N]
            mctT = stb[:, _OFF_MCT:_OFF_MCT + N]
            wband = stb[:, _OFF_WB:_OFF_WB + WBW]
            bj = stf[:, 0:1]
            ones = stf[:, 1:2]
            wrow = stf[:, 2:3]
            nposb = stf[:, 3:4]
            wposb = stf[:, 4:4 + POS_PER_CORE]

            shiftc = st.tile([128, 1], f32, tag="shiftc")
            nc.vector.memset(shiftc[:, :], -EXP_SHIFT)

            # ---- bias matrices: bias^T = Wmc.T @ mcT + bj  ([D, N]) ----
            ps_b = ppp.tile([128, 4 * N], f32, tag="pp")
            nc.tensor.matmul(out=ps_b[:, 0:N], lhsT=Wmc,
                             rhs=mcpT, start=True, stop=False)
            nc.tensor.matmul(out=ps_b[:, N:2 * N], lhsT=Wmc,
                             rhs=mctT, start=False, stop=True)
            biasP = st.tile([D, N], f32, tag="biasP")
            biasT = st.tile([D, N], f32, tag="biasT")
            nc.vector.tensor_scalar_add(biasP[:, :], ps_b[:, 0:N], bj)
            nc.vector.tensor_scalar_add(biasT[:, :], ps_b[:, N:2 * N], bj)
            # doubled copies for per-super adds
            biasP2 = st.tile([D, 2 * N], f32, tag="biasP2")
            biasT2 = st.tile([D, 2 * N], f32, tag="biasT2")
            nc.vector.tensor_copy(biasP2[:, 0:N], biasP[:, :])
            nc.vector.tensor_copy(biasP2[:, N:2 * N], biasP[:, :])
            nc.vector.tensor_copy(biasT2[:, 0:N], biasT[:, :])
            nc.vector.tensor_copy(biasT2[:, N:2 * N], biasT[:, :])

            # diag1 bias part (prologue: depends only on biasP)
            scol = st.tile([128, 4], f32, tag="scol")
            scr2 = st.tile([128, N], f32, tag="scr2")
            bcol = st.tile([128, 1], f32, tag="bcol")
            nc.vector.tensor_tensor(out=scr2[:, :], in0=fT,
                                    in1=biasP[:, :], op=Alu.mult)
            nc.vector.tensor_reduce(out=bcol[:, 0:1], in_=scr2[:, :],
                                    axis=mybir.AxisListType.X, op=Alu.add)
            nc.vector.tensor_tensor(out=scol[:, 1:2], in0=bcol[:, :],
                                    in1=nposb, op=Alu.mult)
            nc.vector.memset(scol[:, 3:4], 0.0)

            # persistent accumulators
            psum_S = accp.tile([128, 2 * N], f32, tag="S")   # row j = lse sums
            psum_G = accp.tile([128, N], f32, tag="G")       # sum_p rawpos_p^T
            diag2buf = st.tile([128, POS_PER_CORE], f32, tag="d2buf")

            # ---- software-pipelined producer: loads + pos/pred + bias for
            # super s (G matmuls are emitted separately, off the critical
            # path) ----
            def stage_a(s):
                lp = locpool.tile([C, 2 * N], bf16, tag="lp")
                lt = locpool.tile([C, 2 * N], bf16, tag="lt")
                nc.sync.dma_start(out=lp[:, :], in_=locp_d[s, :, :])
                nc.sync.dma_start(out=lt[:, :], in_=loct_d[s, :, :])

                pp = ppp.tile([128, 4 * N], f32, tag="pp")
                nc.tensor.matmul(out=pp[:, 0:2 * N], lhsT=Wl,
                                 rhs=lp[:, :], start=True, stop=True)
                nc.tensor.matmul(out=pp[:, 2 * N:4 * N], lhsT=Wl,
                                 rhs=lt[:, :], start=True, stop=True)

                posT2 = work.tile([128, 2 * N], bf16, tag="posT2")
                predT2 = work.tile([128, 2 * N], bf16, tag="predT2")
                nc.vector.tensor_tensor(out=posT2[:, :], in0=pp[:, 0:2 * N],
                                        in1=biasP2[:, :], op=Alu.add)
                nc.vector.tensor_tensor(out=predT2[:, :], in0=pp[:, 2 * N:4 * N],
                                        in1=biasT2[:, :], op=Alu.add)
                return posT2, predT2, lp

            def emit_G(s, lp):
                # G accumulation (raw positive, before bias)
                nc.tensor.matmul(out=psum_G[:, :], lhsT=Wl,
                                 rhs=lp[:, 0:N],
                                 start=(s == 0), stop=False)
                nc.tensor.matmul(out=psum_G[:, :], lhsT=Wl,
                                 rhs=lp[:, N:2 * N],
                                 start=False, stop=(s == N_SUPERS - 1))

            def emit_logits(posT2, predT2, co):
                lg = lgp.tile([128, 4 * N], f32, tag="lg")
                nc.tensor.matmul(out=lg[:, 0:N],
                                 lhsT=posT2[:, co:co + 128],
                                 rhs=fT, start=True, stop=False)
                nc.tensor.matmul(out=lg[:, 2 * N:3 * N],
                                 lhsT=posT2[:, co:co + 128],
                                 rhs=predT2[:, co:co + N],
                                 start=True, stop=False)
                nc.tensor.matmul(out=lg[:, N:2 * N],
                                 lhsT=posT2[:, co + 128:co + 256],
                                 rhs=fT, start=False, stop=True)
                nc.tensor.matmul(out=lg[:, 3 * N:4 * N],
                                 lhsT=posT2[:, co + 128:co + 256],
                                 rhs=predT2[:, co:co + N],
                                 start=False, stop=True)
                return lg

            def emit_exp(lg):
                et = exps.tile([128, 4 * N], bf16, tag="et")
                # exp(l - EXP_SHIFT): keeps sums within ScalarE Ln range;
                # the shift is added back on the host (+2*EXP_SHIFT).
                nc.scalar.activation(et[:, :], lg[:, :], Act.Exp,
                                     bias=shiftc[:, 0:1])
                return et

            def emit_lse(j, et):
                # shifted ones-column band: lhsT = wband[:, 35-j : 163-j]
                # has its all-ones column at output row j, accumulating
                # sum_m exp into psum_S[j, :].
                first = (j == 0)
                last = (j == POS_PER_CORE - 1)
                wb = wband[:, 35 - j:35 - j + 128]
                nc.tensor.matmul(out=psum_S[:, 0:N],
                                 lhsT=wb, rhs=et[:, 0:N],
                                 start=first, stop=False)
                nc.tensor.matmul(out=psum_S[:, 0:N],
                                 lhsT=wb, rhs=et[:, N:2 * N],
                                 start=False, stop=False)
                nc.tensor.matmul(out=psum_S[:, N:2 * N],
                                 lhsT=wb, rhs=et[:, 2 * N:3 * N],
                                 start=False, stop=False)
                nc.tensor.matmul(out=psum_S[:, N:2 * N],
                                 lhsT=wb, rhs=et[:, 3 * N:4 * N],
                                 start=False, stop=last)

            import os
            LSE_LAG = int(os.environ.get("LSE_LAG", "1"))
            pending = []
            staged = stage_a(0)
            emit_G(0, staged[2])
            for s in range(N_SUPERS):
                posT2, predT2, _ = staged

                lg0 = emit_logits(posT2, predT2, 0)
                if s + 1 < N_SUPERS:
                    staged = stage_a(s + 1)
                pending.append((2 * s, emit_exp(lg0)))
                while len(pending) > LSE_LAG:
                    emit_lse(*pending.pop(0))

                # diag2: d2s = posT*predT (gpsimd), segmented-reduce on DVE
                d2s = work.tile([128, 2 * N], f32, tag="d2s")
                nc.gpsimd.tensor_tensor(out=d2s[:, :], in0=posT2[:, :],
                                        in1=predT2[:, :], op=Alu.mult)
                nc.vector.tensor_reduce(
                    out=diag2buf[:, 2 * s:2 * s + 2],
                    in_=d2s[:, :].rearrange("p (k n) -> p k n", k=2),
                    axis=mybir.AxisListType.X, op=Alu.add)

                lg1 = emit_logits(posT2, predT2, N)
                pending.append((2 * s + 1, emit_exp(lg1)))
                while len(pending) > LSE_LAG:
                    emit_lse(*pending.pop(0))

                if s + 1 < N_SUPERS:
                    emit_G(s + 1, staged[2])
            while pending:
                emit_lse(*pending.pop(0))

            # ---- epilogue ----
            J = POS_PER_CORE
            logout = st.tile([128, 2 * N], f32, tag="logout")
            logacc = st.tile([128, 1], f32, tag="logacc")
            nc.scalar.activation(logout[0:J, :], psum_S[0:J, :], Act.Ln,
                                 accum_out=logacc[0:J, 0:1])

            scr1 = st.tile([128, N], f32, tag="scr1")
            # diag1 raw part: <fT, G>
            nc.vector.tensor_tensor(out=scr1[:, :], in0=fT,
                                    in1=psum_G[:, :], op=Alu.mult)
            nc.vector.tensor_reduce(out=scol[:, 0:1], in_=scr1[:, :],
                                    axis=mybir.AxisListType.X, op=Alu.add)
            # diag2 weighted
            wd2 = st.tile([128, POS_PER_CORE], f32, tag="wd2")
            nc.vector.tensor_tensor(out=wd2[:, :], in0=diag2buf[:, :],
                                    in1=wposb, op=Alu.mult)
            nc.vector.tensor_reduce(out=scol[:, 2:3], in_=wd2[:, :],
                                    axis=mybir.AxisListType.X, op=Alu.add)

            # weighted lse total -> psF[64,0]; diag total -> psF[64,1:4]
            psF = lgp.tile([128, 4 * N], f32, tag="lg")
            nc.tensor.matmul(out=psF[64:65, 0:1], lhsT=wrow[0:J, :],
                             rhs=logacc[0:J, :], start=True, stop=False)
            nc.tensor.matmul(out=psF[64:65, 1:4], lhsT=ones,
                             rhs=scol[:, 0:3], start=False, stop=True)
            tmp1 = st.tile([1, 1], f32, tag="tmp1")
            out_sb = st.tile([1, 1], f32, tag="out_sb")
            nc.vector.tensor_reduce(out=tmp1[0:1, :], in_=psF[64:65, 1:4],
                                    axis=mybir.AxisListType.X, op=Alu.add)
            nc.vector.tensor_tensor(out=out_sb[0:1, :], in0=psF[64:65, 0:1],
                                    in1=tmp1[0:1, :], op=Alu.subtract)
            nc.sync.dma_start(out=out_d[:, :], in_=out_sb[0:1, :])

    nc.finalize()
    return nc


def _get_nc():
    global _CACHED_NC
    if _CACHED_NC is None:
        _CACHED_NC = _build_nc()
    return _CACHED_NC


def _core_position_lists():
    """275 positions -> 8 cores: 3 cores x 35, 5 cores x 34."""
    lists = []
    start = 0
    for i in range(N_CORES):
        cnt = 35 if i < 3 else 34
        lists.append(list(range(start, start + cnt)))
        start += cnt
    assert start == P
    return lists


def _prep_in_maps(f_t_global, x_t_local, x_t_prev_local, m_t, m_t_prev, c_t,
                  c_t_prev, W_join, b_join):
    import ml_dtypes
    bf16 = ml_dtypes.bfloat16

    # [N, C, SY, SX] -> [P, C, N]
    locp_full = np.ascontiguousarray(
        x_t_prev_local.reshape(N, C, P).transpose(2, 1, 0))
    loct_full = np.ascontiguousarray(
        x_t_local.reshape(N, C, P).transpose(2, 1, 0))

    # packed bf16 statics [128, STB_COLS]
    stb = np.zeros((128, STB_COLS), dtype=np.float32)
    stb[:, _OFF_WL:_OFF_WL + D] = W_join[:C]
    stb[:, _OFF_WMC:_OFF_WMC + D] = W_join[C:]
    stb[:, _OFF_FT:_OFF_FT + N] = f_t_global.T
    stb[:, _OFF_MCP:_OFF_MCP + N] = np.concatenate([m_t_prev, c_t_prev], 1).T
    stb[:, _OFF_MCT:_OFF_MCT + N] = np.concatenate([m_t, c_t], 1).T
    stb[:, _OFF_WB + 35] = 1.0
    stb = stb.astype(bf16)

    in_maps = []
    for ids in _core_position_lists():
        npos = len(ids)
        locp = np.zeros((POS_PER_CORE, C, N), dtype=np.float32)
        loct = np.zeros((POS_PER_CORE, C, N), dtype=np.float32)
        locp[:npos] = locp_full[ids]
        loct[:npos] = loct_full[ids]
        # [36, C, N] -> [18, C, 2N] (two consecutive positions side by side)
        locp = np.ascontiguousarray(
            locp.reshape(N_SUPERS, 2, C, N).transpose(0, 2, 1, 3)
                .reshape(N_SUPERS, C, 2 * N)).astype(bf16)
        loct = np.ascontiguousarray(
            loct.reshape(N_SUPERS, 2, C, N).transpose(0, 2, 1, 3)
                .reshape(N_SUPERS, C, 2 * N)).astype(bf16)
        # packed f32 statics [128, STF_COLS]: [bj | ones | wrow | nposb | wposb]
        stf = np.zeros((128, STF_COLS), dtype=np.float32)
        stf[:, 0] = b_join.reshape(D)
        stf[:, 1] = 1.0
        stf[:npos, 2] = 1.0
        stf[:, 3] = float(npos)
        stf[:, 4:4 + npos] = 1.0
        in_maps.append({"locp": locp, "loct": loct, "stb": stb, "stf": stf})
    return in_maps


def kernel(f_t_global, x_t_local, x_t_prev_local, m_t, m_t_prev, c_t,
           c_t_prev, W_join, b_join):
    from concourse.bass_utils import run_bass_kernel_spmd

    args = [f_t_global, x_t_local, x_t_prev_local, m_t, m_t_prev, c_t,
            c_t_prev, W_join, b_join]
    args = [np.asarray(a, dtype=np.float32) for a in args]
    in_maps = _prep_in_maps(*args)
    nc = _get_nc()
    res = run_bass_kernel_spmd(nc, in_maps, core_ids=list(range(N_CORES)))
    total = 0.0
    for i in range(N_CORES):
        total += float(res.results[i]["out"][0, 0])
    return np.float32(total / (P * N) + 2.0 * EXP_SHIFT)


# revision 20
# speedup vs baseline: 1.2097x; 1.0525x over previous
"""Distributed Trainium2 kernel for the contrastive InfoNCE loss problem.

Strategy: shard the P = SY*SX = 275 position axis across 8 NeuronCores
(36 position slots per core, zero-padded + weight-masked).  Each core
computes, for its positions p (bf16 matmul pipeline, f32 accumulation):
    pos_p^T  = Wl.T @ locp_p^T + bias_prev^T          [D, N]
    pred_p^T = Wl.T @ loct_p^T + bias_t^T             [D, N]
    logits1^T[m,n] = pos_p[m] . f[n]     (lhsT = pos_p^T tile, rhs = f^T)
    logits2^T[m,n] = pos_p[m] . pred_p[n]
    lse sums via shifted-ones-band matmuls over exp tiles -> one PSUM row
    per position; diagonal terms via a PSUM-accumulated G = sum_p rawpos_p
    (loss1) and a gpsimd multiply + DVE reduce (loss2).
Host sums the 8 per-core scalars and divides by P*N.
"""

import numpy as np

# Problem constants (from the nn_ALL_9320079032780 spec).
N = 256
C = 128
SY, SX = 11, 25
P = SY * SX  # 275
D = 128
DM = 64
DC = 64
N_CORES = 8
POS_PER_CORE = 36  # padded; 18 supers of 2 positions
N_SUPERS = POS_PER_CORE // 2
WBW = 35 + 128  # width of the shifted ones-column band matrix

EXP_SHIFT = 20.0

# packed bf16 statics layout: [Wl | Wmc | fT | mcpT | mctT | wband]
_OFF_WL = 0
_OFF_WMC = _OFF_WL + D
_OFF_FT = _OFF_WMC + D
_OFF_MCP = _OFF_FT + N
_OFF_MCT = _OFF_MCP + N
_OFF_WB = _OFF_MCT + N
STB_COLS = _OFF_WB + WBW
# packed f32 statics layout: [bj | ones | wrow | nposb | wposb]
STF_COLS = 4 + POS_PER_CORE

_CACHED_NC = None


def _build_nc():
    import concourse.bass as bass  # noqa: F401
    import concourse.mybir as mybir
    import concourse.tile as tile
    from concourse import bacc

    f32 = mybir.dt.float32
    bf16 = mybir.dt.bfloat16
    Alu = mybir.AluOpType
    Act = mybir.ActivationFunctionType

    nc = bacc.Bacc("TRN2", target_bir_lowering=False, debug=False,
                   num_devices=N_CORES)

    # Make the act-table pass pick the combined exp+ln set so the kernel
    # pays a single ACT_TABLE_LOAD instead of one per function family.
    from concourse.hw_specs import get_activation_tables
    _tabs = get_activation_tables(nc.m.arch)
    _Exp, _Ln = mybir.ActivationFunctionType.Exp, mybir.ActivationFunctionType.Ln
    for _name, _fns in _tabs.items():
        if _name != "natural_log_exp_and_others":
            _fns.discard(_Exp)
            _fns.discard(_Ln)

    locp_d = nc.declare_dram_parameter("locp", [N_SUPERS, C, 2 * N], bf16, isOutput=False)
    loct_d = nc.declare_dram_parameter("loct", [N_SUPERS, C, 2 * N], bf16, isOutput=False)
    stb_d = nc.declare_dram_parameter("stb", [128, STB_COLS], bf16, isOutput=False)
    stf_d = nc.declare_dram_parameter("stf", [128, STF_COLS], f32, isOutput=False)
    out_d = nc.declare_dram_parameter("out", [1, 1], f32, isOutput=True)

    with tile.TileContext(nc) as tc:
        with (
            tc.tile_pool(name="statics", bufs=1) as st,
            tc.tile_pool(name="loc", bufs=3) as locpool,
            tc.tile_pool(name="work", bufs=3) as work,
            tc.tile_pool(name="exps", bufs=3) as exps,
            tc.tile_pool(name="pp", bufs=1, space="PSUM") as ppp,
            tc.tile_pool(name="lg", bufs=2, space="PSUM") as lgp,
            tc.tile_pool(name="acc", bufs=1, space="PSUM") as accp,
        ):
            # ---- statics: two packed DMAs ----
            stb = st.tile([128, STB_COLS], bf16, tag="stb")
            stf = st.tile([128, STF_COLS], f32, tag="stf")
            nc.sync.dma_start(out=stb[:, :], in_=stb_d[:, :])
            nc.sync.dma_start(out=stf[:, :], in_=stf_d[:, :])
            Wl = stb[:, _OFF_WL:_OFF_WL + D]
            Wmc = stb[:, _OFF_WMC:_OFF_WMC + D]
            fT = stb[:, _OFF_FT:_OFF_FT + N]
            mcpT = stb[:, _OFF_MCP:_OFF_MCP + N]
            mctT = stb[:, _OFF_MCT:_OFF_MCT + N]
            wband = stb[:, _OFF_WB:_OFF_WB + WBW]
            bj = stf[:, 0:1]
            ones = stf[:, 1:2]
            wrow = stf[:, 2:3]
            nposb = stf[:, 3:4]
            wposb = stf[:, 4:4 + POS_PER_CORE]

            shiftc = st.tile([128, 1], f32, tag="shiftc")
            nc.vector.memset(shiftc[:, :], -EXP_SHIFT)

            # ---- bias matrices: bias^T = Wmc.T @ mcT + bj  ([D, N]) ----
            ps_b = ppp.tile([128, 4 * N], f32, tag="pp")
            nc.tensor.matmul(out=ps_b[:, 0:N], lhsT=Wmc,
                             rhs=mcpT, start=True, stop=False)
            nc.tensor.matmul(out=ps_b[:, N:2 * N], lhsT=Wmc,
                             rhs=mctT, start=False, stop=True)
            biasP = st.tile([D, N], f32, tag="biasP")
            biasT = st.tile([D, N], f32, tag="biasT")
            nc.vector.tensor_scalar_add(biasP[:, :], ps_b[:, 0:N], bj)
            nc.vector.tensor_scalar_add(biasT[:, :], ps_b[:, N:2 * N], bj)
            # broadcast views for per-super adds (read twice along a
            # step-0 middle dim instead of materialized copies)
            biasP2b = biasP[:, :].unsqueeze(1).broadcast_to([D, 2, N])
            biasT2b = biasT[:, :].unsqueeze(1).broadcast_to([D, 2, N])

            # diag1 bias part (prologue: depends only on biasP)
            scol = st.tile([128, 4], f32, tag="scol")
            scr2 = st.tile([128, N], f32, tag="scr2")
            bcol = st.tile([128, 1], f32, tag="bcol")
            nc.vector.tensor_tensor(out=scr2[:, :], in0=fT,
                                    in1=biasP[:, :], op=Alu.mult)
            nc.vector.tensor_reduce(out=bcol[:, 0:1], in_=scr2[:, :],
                                    axis=mybir.AxisListType.X, op=Alu.add)
            nc.vector.tensor_tensor(out=scol[:, 1:2], in0=bcol[:, :],
                                    in1=nposb, op=Alu.mult)
            nc.vector.memset(scol[:, 3:4], 0.0)

            # persistent accumulators
            psum_S = accp.tile([128, 2 * N], f32, tag="S")   # row j = lse sums
            psum_G = accp.tile([128, N], f32, tag="G")       # sum_p rawpos_p^T
            diag2buf = st.tile([128, POS_PER_CORE], f32, tag="d2buf")

            # ---- software-pipelined producer: loads + pos/pred + bias for
            # super s (G matmuls are emitted separately, off the critical
            # path) ----
            def stage_a(s):
                lp = locpool.tile([C, 2 * N], bf16, tag="lp")
                lt = locpool.tile([C, 2 * N], bf16, tag="lt")
                nc.sync.dma_start(out=lp[:, :], in_=locp_d[s, :, :])
                nc.sync.dma_start(out=lt[:, :], in_=loct_d[s, :, :])

                pp = ppp.tile([128, 4 * N], f32, tag="pp")
                nc.tensor.matmul(out=pp[:, 0:2 * N], lhsT=Wl,
                                 rhs=lp[:, :], start=True, stop=True)
                nc.tensor.matmul(out=pp[:, 2 * N:4 * N], lhsT=Wl,
                                 rhs=lt[:, :], start=True, stop=True)

                posT2 = work.tile([128, 2 * N], bf16, tag="posT2")
                predT2 = work.tile([128, 2 * N], bf16, tag="predT2")
                nc.vector.tensor_tensor(
                    out=posT2[:, :].rearrange("p (k n) -> p k n", k=2),
                    in0=pp[:, 0:2 * N].rearrange("p (k n) -> p k n", k=2),
                    in1=biasP2b, op=Alu.add)
                nc.vector.tensor_tensor(
                    out=predT2[:, :].rearrange("p (k n) -> p k n", k=2),
                    in0=pp[:, 2 * N:4 * N].rearrange("p (k n) -> p k n", k=2),
                    in1=biasT2b, op=Alu.add)
                return posT2, predT2, lp

            def emit_G(s, lp):
                # G accumulation (raw positive, before bias)
                nc.tensor.matmul(out=psum_G[:, :], lhsT=Wl,
                                 rhs=lp[:, 0:N],
                                 start=(s == 0), stop=False)
                nc.tensor.matmul(out=psum_G[:, :], lhsT=Wl,
                                 rhs=lp[:, N:2 * N],
                                 start=False, stop=(s == N_SUPERS - 1))

            def emit_logits(posT2, predT2, co):
                lg = lgp.tile([128, 4 * N], f32, tag="lg")
                nc.tensor.matmul(out=lg[:, 0:N],
                                 lhsT=posT2[:, co:co + 128],
                                 rhs=fT, start=True, stop=False)
                nc.tensor.matmul(out=lg[:, 2 * N:3 * N],
                                 lhsT=posT2[:, co:co + 128],
                                 rhs=predT2[:, co:co + N],
                                 start=True, stop=False)
                nc.tensor.matmul(out=lg[:, N:2 * N],
                                 lhsT=posT2[:, co + 128:co + 256],
                                 rhs=fT, start=False, stop=True)
                nc.tensor.matmul(out=lg[:, 3 * N:4 * N],
                                 lhsT=posT2[:, co + 128:co + 256],
                                 rhs=predT2[:, co:co + N],
                                 start=False, stop=True)
                return lg

            def emit_exp(lg):
                et = exps.tile([128, 4 * N], bf16, tag="et")
                # exp(l - EXP_SHIFT): keeps sums within ScalarE Ln range;
                # the shift is added back on the host (+2*EXP_SHIFT).
                nc.scalar.activation(et[:, :], lg[:, :], Act.Exp,
                                     bias=shiftc[:, 0:1])
                return et

            def emit_lse(j, et):
                # shifted ones-column band: lhsT = wband[:, 35-j : 163-j]
                # has its all-ones column at output row j, accumulating
                # sum_m exp into psum_S[j, :].
                first = (j == 0)
                last = (j == POS_PER_CORE - 1)
                wb = wband[:, 35 - j:35 - j + 128]
                nc.tensor.matmul(out=psum_S[:, 0:N],
                                 lhsT=wb, rhs=et[:, 0:N],
                                 start=first, stop=False)
                nc.tensor.matmul(out=psum_S[:, 0:N],
                                 lhsT=wb, rhs=et[:, N:2 * N],
                                 start=False, stop=False)
                nc.tensor.matmul(out=psum_S[:, N:2 * N],
                                 lhsT=wb, rhs=et[:, 2 * N:3 * N],
                                 start=False, stop=False)
                nc.tensor.matmul(out=psum_S[:, N:2 * N],
                                 lhsT=wb, rhs=et[:, 3 * N:4 * N],
                                 start=False, stop=last)

            import os
            LSE_LAG = int(os.environ.get("LSE_LAG", "3"))
            pending = []
            staged = stage_a(0)
            emit_G(0, staged[2])
            for s in range(N_SUPERS):
                posT2, predT2, _ = staged

                lg0 = emit_logits(posT2, predT2, 0)
                if s + 1 < N_SUPERS:
                    staged = stage_a(s + 1)
                pending.append((2 * s, emit_exp(lg0)))
                while len(pending) > LSE_LAG:
                    emit_lse(*pending.pop(0))

                # diag2: d2s = posT*predT (gpsimd), segmented-reduce on DVE
                d2s = work.tile([128, 2 * N], f32, tag="d2s")
                nc.gpsimd.tensor_tensor(out=d2s[:, :], in0=posT2[:, :],
                                        in1=predT2[:, :], op=Alu.mult)
                nc.vector.tensor_reduce(
                    out=diag2buf[:, 2 * s:2 * s + 2],
                    in_=d2s[:, :].rearrange("p (k n) -> p k n", k=2),
                    axis=mybir.AxisListType.X, op=Alu.add)

                lg1 = emit_logits(posT2, predT2, N)
                pending.append((2 * s + 1, emit_exp(lg1)))
                while len(pending) > LSE_LAG:
                    emit_lse(*pending.pop(0))

                if s + 1 < N_SUPERS:
                    emit_G(s + 1, staged[2])
            while pending:
                emit_lse(*pending.pop(0))

            # ---- epilogue ----
            J = POS_PER_CORE
            logout = st.tile([128, 2 * N], f32, tag="logout")
            logacc = st.tile([128, 1], f32, tag="logacc")
            nc.scalar.activation(logout[0:J, :], psum_S[0:J, :], Act.Ln,
                                 accum_out=logacc[0:J, 0:1])

            scr1 = st.tile([128, N], f32, tag="scr1")
            # diag1 raw part: <fT, G>
            nc.vector.tensor_tensor(out=scr1[:, :], in0=fT,
                                    in1=psum_G[:, :], op=Alu.mult)
            nc.vector.tensor_reduce(out=scol[:, 0:1], in_=scr1[:, :],
                                    axis=mybir.AxisListType.X, op=Alu.add)
            # diag2 weighted
            wd2 = st.tile([128, POS_PER_CORE], f32, tag="wd2")
            nc.vector.tensor_tensor(out=wd2[:, :], in0=diag2buf[:, :],
                                    in1=wposb, op=Alu.mult)
            nc.vector.tensor_reduce(out=scol[:, 2:3], in_=wd2[:, :],
                                    axis=mybir.AxisListType.X, op=Alu.add)

            # weighted lse total -> psF[64,0]; diag total -> psF[64,1:4]
            psF = lgp.tile([128, 4 * N], f32, tag="lg")
            nc.tensor.matmul(out=psF[64:65, 0:1], lhsT=wrow[0:J, :],
                             rhs=logacc[0:J, :], start=True, stop=False)
            nc.tensor.matmul(out=psF[64:65, 1:4], lhsT=ones,
                             rhs=scol[:, 0:3], start=False, stop=True)
            tmp1 = st.tile([1, 1], f32, tag="tmp1")
            out_sb = st.tile([1, 1], f32, tag="out_sb")
            nc.vector.tensor_reduce(out=tmp1[0:1, :], in_=psF[64:65, 1:4],
                                    axis=mybir.AxisListType.X, op=Alu.add)
            nc.vector.tensor_tensor(out=out_sb[0:1, :], in0=psF[64:65, 0:1],
                                    in1=tmp1[0:1, :], op=Alu.subtract)
            nc.sync.dma_start(out=out_d[:, :], in_=out_sb[0:1, :])

    nc.finalize()
    return nc


def _get_nc():
    global _CACHED_NC
    if _CACHED_NC is None:
        _CACHED_NC = _build_nc()
    return _CACHED_NC


def _core_position_lists():
    """275 positions -> 8 cores: 3 cores x 35, 5 cores x 34."""
    lists = []
    start = 0
    for i in range(N_CORES):
        cnt = 35 if i < 3 else 34
        lists.append(list(range(start, start + cnt)))
        start += cnt
    assert start == P
    return lists


def _prep_in_maps(f_t_global, x_t_local, x_t_prev_local, m_t, m_t_prev, c_t,
                  c_t_prev, W_join, b_join):
    import ml_dtypes
    bf16 = ml_dtypes.bfloat16

    # [N, C, SY, SX] -> [P, C, N]
    locp_full = np.ascontiguousarray(
        x_t_prev_local.reshape(N, C, P).transpose(2, 1, 0))
    loct_full = np.ascontiguousarray(
        x_t_local.reshape(N, C, P).transpose(2, 1, 0))

    # packed bf16 statics [128, STB_COLS]
    stb = np.zeros((128, STB_COLS), dtype=np.float32)
    stb[:, _OFF_WL:_OFF_WL + D] = W_join[:C]
    stb[:, _OFF_WMC:_OFF_WMC + D] = W_join[C:]
    stb[:, _OFF_FT:_OFF_FT + N] = f_t_global.T
    stb[:, _OFF_MCP:_OFF_MCP + N] = np.concatenate([m_t_prev, c_t_prev], 1).T
    stb[:, _OFF_MCT:_OFF_MCT + N] = np.concatenate([m_t, c_t], 1).T
    stb[:, _OFF_WB + 35] = 1.0
    stb = stb.astype(bf16)

    in_maps = []
    for ids in _core_position_lists():
        npos = len(ids)
        locp = np.zeros((POS_PER_CORE, C, N), dtype=np.float32)
        loct = np.zeros((POS_PER_CORE, C, N), dtype=np.float32)
        locp[:npos] = locp_full[ids]
        loct[:npos] = loct_full[ids]
        # [36, C, N] -> [18, C, 2N] (two consecutive positions side by side)
        locp = np.ascontiguousarray(
            locp.reshape(N_SUPERS, 2, C, N).transpose(0, 2, 1, 3)
                .reshape(N_SUPERS, C, 2 * N)).astype(bf16)
        loct = np.ascontiguousarray(
            loct.reshape(N_SUPERS, 2, C, N).transpose(0, 2, 1, 3)
                .reshape(N_SUPERS, C, 2 * N)).astype(bf16)
        # packed f32 statics [128, STF_COLS]: [bj | ones | wrow | nposb | wposb]
        stf = np.zeros((128, STF_COLS), dtype=np.float32)
        stf[:, 0] = b_join.reshape(D)
        stf[:, 1] = 1.0
        stf[:npos, 2] = 1.0
        stf[:, 3] = float(npos)
        stf[:, 4:4 + npos] = 1.0
        in_maps.append({"locp": locp, "loct": loct, "stb": stb, "stf": stf})
    return in_maps


def kernel(f_t_global, x_t_local, x_t_prev_local, m_t, m_t_prev, c_t,
           c_t_prev, W_join, b_join):
    from concourse.bass_utils import run_bass_kernel_spmd

    args = [f_t_global, x_t_local, x_t_prev_local, m_t, m_t_prev, c_t,
            c_t_prev, W_join, b_join]
    args = [np.asarray(a, dtype=np.float32) for a in args]
    in_maps = _prep_in_maps(*args)
    nc = _get_nc()
    res = run_bass_kernel_spmd(nc, in_maps, core_ids=list(range(N_CORES)))
    total = 0.0
    for i in range(N_CORES):
        total += float(res.results[i]["out"][0, 0])
    return np.float32(total / (P * N) + 2.0 * EXP_SHIFT)
